# revision 2
# baseline (speedup 1.0000x reference)
"""AttentionPool Trainium2 kernel.

Computes, for x [B, N, D], mask [B, N], q [D]:
    logits = einsum('bnd,d->bn', x, q);  logits[~mask] = -inf
    w = softmax(logits, axis=-1)
    out = einsum('bn,bnd->bd', w, x)

Sharding: data-parallel over B across 8 NeuronCores (4 rows per core).

Position enumeration (per row): n = t8*1024 + p*8 + s, with p = SBUF
partition, s in [0,8), t8 in [0,8). Each partition reads 8 consecutive
positions = 8 KiB contiguous DRAM per (p, t8) -> one fat DMA descriptor.
A "tile" is (t8, s): 128 positions, one per partition; col = t8*8 + s.

Per-core device program, fully chunk-pipelined (per batch row, 8 chunks):
  - DMA chunk c into SBUF (f32); ScalarE casts it to bf16 (for pass 2).
  - Logits on DVE via a custom scan op (registered in-process; ships its own
    uop tables in the NEFF — the stock fused-reduce opcodes crash this
    terminal's ucode): one op per chunk computes the running prefix of x*q
    over 2048 elements; a stride-0 output AP keeps only each 256-element
    segment end -> 8 segment dot-products per op at ~1.09 cycles/element.
    Tile logits = adjacent-difference of segment ends (+ mask bias).
  - Softmax shift: from chunk 0 only (DVE reduce_max + GPSIMD
    partition_all_reduce(max), -10 margin). The host divides by Z, so any
    shift cancels exactly; it only must be within ~80 of the true row max
    to avoid fp32 overflow/underflow. This removes the whole-row barrier,
    so exp and pass 2 run per chunk, overlapped with the DMA stream.
  - w = exp(logits - shift) on ScalarE (bf16 out), accum_out -> per-chunk
    partition exp-sums (f32); Z summed on host.
  - Pass 2 on TensorE in bf16, M=2: lhsT = two w columns [128, 2], rhs =
    their two x tiles side by side [128, 512] (LDWEIGHTS ~2 cycles), single
    PSUM accumulation chain [2, 512]. Row result = acc[0, 0:256] +
    acc[1, 256:512]; the off-diagonal cross blocks are discarded on host.
  - Host combines the halves and divides by Z.

USE_BF16_PASS2=False switches pass 2 (and w) to fp32: ~30% slower end to
end, final relative error ~2e-5 instead of ~4e-3.
"""

import numpy as np

B, N, D = 32, 8192, 256
N_CORES = 8
B_LOC = B // N_CORES  # 4
P = 128
S = 8               # consecutive positions per partition (8 KiB descriptors)
T8 = N // (P * S)   # 8 chunk groups per row
T = N // P          # 64 tiles (columns) per row
NCHUNK = T8         # one DMA chunk per t8 group
GK = 9              # ends layout: 1 zero col + 8 segment ends per chunk

USE_BF16_PASS2 = True

_cache = {}

_SCAN_OP_NAME = "ATTNPOOL_MUL_SCAN"


def _register_scan_op():
    """Register a custom DVE op computing scan(add, Src0*Src1) in-process.

    The stock TENSOR_TENSOR_REDUCE / TENSOR_TENSOR_SCAN opcodes crash this
    terminal's ucode; custom-DVE ops ship their own uop tables inside the
    NEFF, so they are self-contained.
    """
    from concourse import dve_ops
    from concourse.dve_spec import AluOp, Spec, Src0, Src1, scan, lower, _has_src1
    from concourse.dve_uop import DveOpSpec

    for op in dve_ops.OPS:
        if op.name == _SCAN_OP_NAME:
            return op
    spec = Spec(
        body=scan(AluOp.ADD, Src0 * Src1),
        reference=lambda in0, in1, c0, c1, c2: np.cumsum(
            in0.astype(np.float32) * in1, axis=1, dtype=np.float32
        ),
    )
    row = dve_ops._CUSTOM_DVE_ROW_BASE + len(dve_ops.OPS)
    assert row < 0x20
    shas = {}
    for ver in ("v3", "v4"):
        tmp = DveOpSpec(
            name=_SCAN_OP_NAME,
            opcode=row,
            uops=lower(spec, ver=ver),
            rd1_en=_has_src1(spec),
        )
        shas[ver] = tmp.sha(ver)
    op = dve_ops.DveOp(_SCAN_OP_NAME, spec, subdim=False, uops_sha=shas)
    dve_ops.OPS.append(op)
    dve_ops._SUB_OPCODE_FOR_NAME[_SCAN_OP_NAME] = row
    dve_ops.CUSTOM_DVE_SPECS[_SCAN_OP_NAME] = spec
    return op


def _build():
    import concourse.bass as bass
    import concourse.tile as tile
    from concourse import bacc, mybir, bass_isa

    scan_op = _register_scan_op()

    dt = mybir.dt
    nc = bacc.Bacc(
        "TRN2", target_bir_lowering=False, debug=False, num_devices=N_CORES
    )
    x_d = nc.dram_tensor("x", [B_LOC, N, D], dt.float32, kind="ExternalInput").ap()
    bias_d = nc.dram_tensor(
        "bias", [B_LOC, P, T], dt.float32, kind="ExternalInput"
    ).ap()
    q_d = nc.dram_tensor("q", [P, D], dt.float32, kind="ExternalInput").ap()
    out_d = nc.dram_tensor(
        "out", [B_LOC, 2, 2 * D], dt.float32, kind="ExternalOutput"
    ).ap()
    z_d = nc.dram_tensor("z", [B_LOC, P, NCHUNK], dt.float32, kind="ExternalOutput").ap()

    wdt = dt.bfloat16 if USE_BF16_PASS2 else dt.float32

    with tile.TileContext(nc) as tc:
        with (
            tc.tile_pool(name="singles", bufs=1) as singles,
            tc.tile_pool(name="xf32", bufs=16) as xf32,
            tc.tile_pool(name="xbf", bufs=6) as xbf,
            tc.tile_pool(name="small", bufs=2) as small,
            tc.tile_pool(name="psum", bufs=2, space="PSUM") as psum,
        ):
            qb = singles.tile([P, D], dt.float32)
            nc.scalar.dma_start(qb[:], q_d[:])
            q3 = qb.rearrange("p (u d) -> p u d", u=1).broadcast_to([P, S, D])

            # segment-end accumulator: per chunk group, col 9c = 0 (set once),
            # cols 9c+1..9c+8 = running prefix at each 256-elem segment end.
            ends9 = singles.tile([P, NCHUNK * GK], dt.float32)
            nc.vector.memset(ends9[:], 0.0)

            for b in range(B_LOC):
                bias_t = small.tile([P, T], dt.float32)
                nc.scalar.dma_start(bias_t[:], bias_d[b])

                xrow = x_d[b].rearrange("(t8 p s) d -> p t8 s d", p=P, s=S)
                chunks = []
                bchunks = []
                for c in range(NCHUNK):
                    ch = xf32.tile([P, S, D], dt.float32)
                    nc.sync.dma_start(ch[:], xrow[:, c])
                    chunks.append(ch)
                    if USE_BF16_PASS2:
                        cb = xbf.tile([P, S, D], dt.bfloat16)
                        nc.scalar.copy(cb[:], ch[:])
                        bchunks.append(cb)
                    else:
                        bchunks.append(ch)

                logits = small.tile([P, T], dt.float32)
                w = small.tile([P, T], wdt)
                z8 = small.tile([P, NCHUNK], dt.float32)
                negm = small.tile([P, 1], dt.float32)
                acc = psum.tile([2, 2 * D], dt.float32)
                e9 = ends9.rearrange("p (g k) -> p g k", k=GK)
                l3 = logits.rearrange("p (c k) -> p c k", k=S)

                # per-chunk pipeline: scan -> tile sums -> exp -> matmuls.
                # The softmax shift comes from chunk 0 only: the host divides
                # by Z so any shift cancels exactly; it only needs to be
                # within ~80 of the true row max to avoid overflow/underflow.
                for c in range(NCHUNK):
                    o3 = (
                        ends9[:, c * GK + 1 : c * GK + 1 + S]
                        .rearrange("p (g u) -> p g u", u=1)
                        .broadcast_to([P, S, D])
                    )
                    nc.vector._custom_dve(
                        scan_op,
                        out=o3,
                        in0=chunks[c].rearrange("p s d -> p (s d)"),
                        in1=q3,
                    )
                    # tile sums = adjacent difference of segment ends, + bias
                    nc.vector.tensor_tensor(
                        l3[:, c : c + 1, :],
                        e9[:, c : c + 1, 1 : 1 + S],
                        e9[:, c : c + 1, 0:S],
                        op=mybir.AluOpType.subtract,
                    )
                    nc.vector.tensor_tensor(
                        logits[:, c * S : (c + 1) * S],
                        logits[:, c * S : (c + 1) * S],
                        bias_t[:, c * S : (c + 1) * S],
                        op=mybir.AluOpType.add,
                    )
                    if c == 0:
                        m = small.tile([P, 1], dt.float32)
                        nc.vector.reduce_max(
                            m[:], logits[:, 0:S], axis=mybir.AxisListType.X
                        )
                        mall = small.tile([P, 1], dt.float32)
                        nc.gpsimd.partition_all_reduce(
                            mall[:], m[:], channels=P,
                            reduce_op=bass_isa.ReduceOp.max,
                        )
                        # negm = -(chunk0 max) - 10 (margin)
                        nc.gpsimd.tensor_scalar(
                            negm[:], mall[:], -1.0, -10.0,
                            op0=mybir.AluOpType.mult,
                            op1=mybir.AluOpType.add,
                        )
                    nc.scalar.activation(
                        w[:, c * S : (c + 1) * S],
                        logits[:, c * S : (c + 1) * S],
                        mybir.ActivationFunctionType.Exp,
                        bias=negm[:],
                        accum_out=z8[:, c : c + 1],
                    )
                    # pass 2, M=2: lhsT = two w columns [128, 2], rhs = their
                    # two x tiles side by side [128, 512]. Row result =
                    # acc[0, 0:256] + acc[1, 256:512] (combined on host);
                    # off-diagonal blocks are unused cross terms.
                    cb = bchunks[c]
                    for sp in range(0, S, 2):
                        col = c * S + sp
                        nc.tensor.matmul(
                            acc[:],
                            w[:, col : col + 2],
                            cb[:, sp : sp + 2, :].rearrange("p s d -> p (s d)"),
                            start=(col == 0),
                            stop=(col == T - 2),
                        )
                nc.scalar.dma_start(z_d[b], z8[:])

                halves = small.tile([2, 2 * D], dt.float32)
                nc.scalar.copy(halves[:], acc[:])
                nc.scalar.dma_start(out_d[b], halves[:])

    nc.compile()
    return nc


def _prep_core_inputs(x, mask, q):
    """Host-side shard prep. Returns list of per-core input dicts."""
    qb = np.ascontiguousarray(np.broadcast_to(q[None, :], (P, D)), dtype=np.float32)
    # bias[b, p, col] for col = t8*8 + s, position n = t8*1024 + p*8 + s
    bias_all = np.where(mask, np.float32(0.0), np.float32(-1e30)).astype(np.float32)
    bias_all = bias_all.reshape(B, T8, P, S).transpose(0, 2, 1, 3).reshape(B, P, T)
    in_maps = []
    for i in range(N_CORES):
        sl = slice(i * B_LOC, (i + 1) * B_LOC)
        in_maps.append(
            {
                "x": np.ascontiguousarray(x[sl]),
                "bias": np.ascontiguousarray(bias_all[sl]),
                "q": qb,
            }
        )
    return in_maps


def kernel(x, mask, q, _trace=False, _tmpdir=None):
    from concourse.bass_utils import run_bass_kernel_spmd

    x = np.asarray(x, dtype=np.float32)
    mask = np.asarray(mask)
    q = np.asarray(q, dtype=np.float32)
    assert x.shape == (B, N, D) and mask.shape == (B, N) and q.shape == (D,)

    if "nc" not in _cache:
        _cache["nc"] = _build()
    nc = _cache["nc"]

    in_maps = _prep_core_inputs(x, mask, q)
    res = run_bass_kernel_spmd(
        nc, in_maps, list(range(N_CORES)), trace=_trace, tmpdir=_tmpdir
    )
    out = np.empty((B, D), dtype=np.float32)
    for i in range(N_CORES):
        h = res.results[i]["out"]  # [B_LOC, 2, 512] PSUM halves, unnormalized
        o = h[:, 0, 0:D] + h[:, 1, D : 2 * D]
        z = res.results[i]["z"].astype(np.float64).sum(axis=(1, 2))  # [B_LOC]
        out[i * B_LOC : (i + 1) * B_LOC] = o / z[:, None]
    if _trace:
        return out, res
    return out

